# revision 12
# baseline (speedup 1.0000x reference)
"""MLA attention distributed over 8 TRN2 NeuronCores.

Sharding: tensor-parallel over heads (4 head-groups) x data-parallel over
batch (2). Each core computes, for its (batch, head-group):
  - the shared low-rank compressions c_kv/c_q and the rope key (replicated
    within a batch group),
  - K/V/Q up-projections for its 4 heads,
  - full attention for its 4 heads over all 2048 query positions,
  - a partial output projection (its heads' rows of W_O).
Host gather sums the 4 partial outputs per batch (row-parallel unshard).

All big GEMMs run on the PE in float32r (tf32-class precision, full speed at
N=512). Attention probabilities and V run in bf16; softmax is computed
without max-subtraction (scores are bounded ~|2| at this problem's scale) and
the denominator comes free from a ones-column appended to V.

Layout trick: everything downstream of x contracts over D, which must sit on
the partition dim, so the host feeds x[b].T. All projections are computed
directly in transposed layout [feature, seq]; rope is applied in transposed
layout using host-prepared sin/cos tables and an even/odd column permutation
baked into W_KR/W_QR.
"""

from contextlib import ExitStack

import numpy as np

import concourse.bacc as bacc
import concourse.mybir as mybir
import concourse.tile as tile
from concourse.bass_utils import run_bass_kernel_spmd
from concourse.masks import make_identity

B, L, D, H, DC, DH = 2, 2048, 2048, 16, 512, 128
HG = 4                 # head groups (tensor-parallel degree per batch)
HL = H // HG           # heads per core
HDL = HL * DH          # 512 head-dims per core
P = 128
N1 = 512               # matmul free-dim chunk
F32 = mybir.dt.float32
BF16 = mybir.dt.bfloat16
F32R = mybir.dt.float32r
SCALE = 1.0 / float(np.sqrt(2 * DH))
M1 = 2 * DC + DH       # 1152: [W_DKV | W_DQ | W_KR] fused output rows
MT1 = M1 // P          # 9
KT1 = D // P           # 16
NCH = L // N1          # 4
KT3 = DC // P          # 4
KB = L // P            # 16 key blocks
NEG = -30000.0         # additive mask bias for masked-out keys


def build_nc():
    nc = bacc.Bacc(None, target_bir_lowering=False)

    xT = nc.dram_tensor("xT", [D, L], F32R, kind="ExternalInput")
    w1 = nc.dram_tensor("w1", [D, M1], F32R, kind="ExternalInput")
    wuk = nc.dram_tensor("wuk", [DC, HDL], F32R, kind="ExternalInput")
    w3q = nc.dram_tensor("w3q", [DC, HDL + DH], F32R, kind="ExternalInput")
    wuv = nc.dram_tensor("wuv", [DC, HDL], F32R, kind="ExternalInput")
    wo = nc.dram_tensor("wo", [HDL, D], F32R, kind="ExternalInput")
    cos_d = nc.dram_tensor("cosT", [DH // 2, L], F32, kind="ExternalInput")
    sin_d = nc.dram_tensor("sinT", [DH // 2, L], F32, kind="ExternalInput")
    mask_d = nc.dram_tensor("maskb", [P, KB], F32, kind="ExternalInput")
    out_d = nc.dram_tensor("out", [L, D], F32, kind="ExternalOutput")

    with tile.TileContext(nc) as tc, ExitStack() as es:
        # ---------- constant + psum pools (live whole kernel) ----------
        p_const = es.enter_context(tc.tile_pool(name="const", bufs=1))
        p_ps_g = es.enter_context(tc.tile_pool(name="psg", bufs=2, space="PSUM"))
        p_ps_sc = es.enter_context(tc.tile_pool(name="pssc", bufs=3, space="PSUM"))
        p_ps_av = es.enter_context(tc.tile_pool(name="psav", bufs=2, space="PSUM"))
        p_ps_tp = es.enter_context(tc.tile_pool(name="pstp", bufs=1, space="PSUM"))

        p_tab = es.enter_context(tc.tile_pool(name="tabp", bufs=1, side="right"))
        p_rope = es.enter_context(tc.tile_pool(name="ropep", bufs=1, side="right"))
        cos_t = p_tab.tile([DH // 2, L], F32, name="cos_t")
        sin_t = p_tab.tile([DH // 2, L], F32, name="sin_t")
        nc.sync.dma_start(cos_t[:], cos_d[:])
        nc.sync.dma_start(sin_t[:], sin_d[:])
        bias_t = p_const.tile([P, KB], F32, name="bias_t")
        nc.sync.dma_start(bias_t[:], mask_d[:])
        ident = p_const.tile([P, P], BF16, name="ident")
        make_identity(nc, ident[:])

        # ---------- phase-1 residents ----------
        es_ckv = ExitStack()
        p_ckv = es_ckv.enter_context(tc.tile_pool(name="ckvp", bufs=1))
        es_cq = ExitStack()
        p_cq = es_cq.enter_context(tc.tile_pool(name="cqp", bufs=1))
        es_xrk = ExitStack()
        p_xrk = es_xrk.enter_context(tc.tile_pool(name="xrkp", bufs=1))

        ckv_t = [p_ckv.tile([P, L], F32R, name=f"ckv{i}", tag=f"ckv{i}")
                 for i in range(KT3)]
        cq_t = [p_cq.tile([P, L], F32R, name=f"cq{i}", tag=f"cq{i}")
                for i in range(KT3)]
        xrk_t = p_xrk.tile([P, L], F32, name="xrkT")

        # ---------- phase 1: c_kvT | c_qT | xrkT = [Wdkv|Wdq|Wkr].T @ x.T ----
        es_w1 = ExitStack()
        p_w1 = es_w1.enter_context(tc.tile_pool(name="w1p", bufs=1))
        es_xn = ExitStack()
        p_xn = es_xn.enter_context(tc.tile_pool(name="xnp", bufs=16))

        # interleave the first x-chunk's DMAs with the weight-cache DMAs so
        # the PE k-loop can start as soon as (w1_0, xn_0_0) land
        w1_t = []
        xts0 = []
        for kt in range(KT1):
            t = p_xn.tile([P, N1], F32R, tag="xn", name=f"xn_0_{kt}")
            nc.sync.dma_start(t[:], xT[kt * P:(kt + 1) * P, 0:N1])
            xts0.append(t)
            t = p_w1.tile([P, M1], F32R, name=f"w1_{kt}", tag=f"w1_{kt}")
            nc.sync.dma_start(t[:], w1[kt * P:(kt + 1) * P, :])
            w1_t.append(t)

        dest1 = ckv_t + cq_t + [xrk_t]
        for nci in range(NCH):
            if nci == 0:
                xts = xts0
            else:
                xts = []
                for kt in range(KT1):
                    t = p_xn.tile([P, N1], F32R, tag="xn", name=f"xn_{nci}_{kt}")
                    nc.sync.dma_start(t[:], xT[kt * P:(kt + 1) * P,
                                                nci * N1:(nci + 1) * N1])
                    xts.append(t)
            for mt in range(MT1):
                ps = p_ps_g.tile([P, N1], F32, tag="g", name=f"ps1_{nci}_{mt}")
                for kt in range(KT1):
                    nc.tensor.matmul(ps[:], w1_t[kt][:, mt * P:(mt + 1) * P],
                                     xts[kt][:],
                                     start=(kt == 0), stop=(kt == KT1 - 1))
                nc.vector.tensor_copy(dest1[mt][:, nci * N1:(nci + 1) * N1], ps[:])
        es_xn.close()
        es_w1.close()

        # ---------- long-lived attention inputs (right-side stack) ----------
        es_krqr = ExitStack()
        p_krqr = es_krqr.enter_context(tc.tile_pool(name="krqrp", bufs=1, side="right"))
        kr_t = p_krqr.tile([P, L], F32R, name="krT", tag="krT")
        qr_t = p_krqr.tile([P, L], F32R, name="qrT", tag="qrT")
        es_qc = ExitStack()
        p_qc = es_qc.enter_context(tc.tile_pool(name="qcp", bufs=1, side="right"))
        qc_t = [p_qc.tile([P, L], F32R, tag=f"qc{i}", name=f"qc{i}")
                for i in range(HL)]

        # rope in transposed layout, emitted after the next GEMM phase so
        # its DVE work drains behind that phase's psum evictions.
        def rope_T(src_t, dst_t, pfx):
            # src rows 0:64 = even components, 64:128 = odd (host permuted W)
            for ch in range(NCH):
                cs = slice(ch * N1, (ch + 1) * N1)
                xo = p_rope.tile([64, N1], F32, tag="rxo", name=f"{pfx}xo{ch}")
                nc.sync.dma_start(xo[:], src_t[64:128, cs])
                t1 = p_rope.tile([64, N1], F32, tag="rt1", name=f"{pfx}t1{ch}")
                t2 = p_rope.tile([64, N1], F32, tag="rt2", name=f"{pfx}t2{ch}")
                h2 = p_rope.tile([64, N1], F32R, tag="rh2", name=f"{pfx}h2{ch}")
                xe = src_t[0:64, cs]
                cc, ss = cos_t[:, cs], sin_t[:, cs]
                nc.vector.tensor_tensor(t1[:], xe, cc, mybir.AluOpType.mult)
                nc.vector.tensor_tensor(t2[:], xo[:], ss, mybir.AluOpType.mult)
                nc.vector.tensor_tensor(dst_t[0:64, cs], t1[:], t2[:],
                                        mybir.AluOpType.subtract)
                t3 = p_rope.tile([64, N1], F32, tag="rt1", name=f"{pfx}t3{ch}")
                t4 = p_rope.tile([64, N1], F32, tag="rt2", name=f"{pfx}t4{ch}")
                nc.vector.tensor_tensor(t3[:], xe, ss, mybir.AluOpType.mult)
                nc.vector.tensor_tensor(t4[:], xo[:], cc, mybir.AluOpType.mult)
                nc.vector.tensor_tensor(h2[:], t3[:], t4[:], mybir.AluOpType.add)
                nc.sync.dma_start(dst_t[64:128, cs], h2[:])

        # ---------- phase 3q: q_cT | xrqT = [Wuq_hg|Wqr].T @ c_qT ----------
        es_w3q = ExitStack()
        p_w3q = es_w3q.enter_context(tc.tile_pool(name="w3qp", bufs=1))
        es_xrq = ExitStack()
        p_xrq = es_xrq.enter_context(tc.tile_pool(name="xrqp", bufs=1, side="right"))

        w3q_t = []
        for kt in range(KT3):
            t = p_w3q.tile([P, HDL + DH], F32R, tag=f"w3q{kt}", name=f"w3q{kt}")
            nc.sync.dma_start(t[:], w3q[kt * P:(kt + 1) * P, :])
            w3q_t.append(t)
        xrq_t = p_xrq.tile([P, L], F32, name="xrqT")
        dest3 = qc_t + [xrq_t]
        for nci in range(NCH):
            for mt in range(HL + 1):
                ps = p_ps_g.tile([P, N1], F32, tag="g", name=f"ps3_{nci}_{mt}")
                for kt in range(KT3):
                    nc.tensor.matmul(ps[:], w3q_t[kt][:, mt * P:(mt + 1) * P],
                                     cq_t[kt][:, nci * N1:(nci + 1) * N1],
                                     start=(kt == 0), stop=(kt == KT3 - 1))
                nc.vector.tensor_copy(dest3[mt][:, nci * N1:(nci + 1) * N1], ps[:])
        es_w3q.close()

        # rope-k, emitted here so its DVE work overlaps the 3k matmuls
        rope_T(xrk_t, kr_t, "k")
        es_xrk.close()
        es_cq.close()

        # rope-q, emitted here so its DVE work overlaps the 3k/3v matmuls
        rope_T(xrq_t, qr_t, "q")
        es_xrq.close()

        # ---------- phase 3k: k_cT = Wuk_hg.T @ c_kvT ----------
        es_kc = ExitStack()
        p_kc = es_kc.enter_context(tc.tile_pool(name="kcp", bufs=1, side="right"))
        kc_t = [p_kc.tile([P, L], F32R, tag=f"kc{i}", name=f"kc{i}")
                for i in range(HL)]
        es_wuk = ExitStack()
        p_wuk = es_wuk.enter_context(tc.tile_pool(name="wukp", bufs=1))
        wuk_t = []
        for kt in range(KT3):
            t = p_wuk.tile([P, HDL], F32R, tag=f"wuk{kt}", name=f"wuk{kt}")
            nc.sync.dma_start(t[:], wuk[kt * P:(kt + 1) * P, :])
            wuk_t.append(t)
        for nci in range(NCH):
            for mt in range(HL):
                ps = p_ps_g.tile([P, N1], F32, tag="g", name=f"ps3k_{nci}_{mt}")
                for kt in range(KT3):
                    nc.tensor.matmul(ps[:], wuk_t[kt][:, mt * P:(mt + 1) * P],
                                     ckv_t[kt][:, nci * N1:(nci + 1) * N1],
                                     start=(kt == 0), stop=(kt == KT3 - 1))
                nc.vector.tensor_copy(kc_t[mt][:, nci * N1:(nci + 1) * N1], ps[:])
        es_wuk.close()

        # ---------- phase 3v: v = c_kv @ Wuv_hg (natural), bf16 + ones col ---
        es_wuv = ExitStack()
        p_wuv = es_wuv.enter_context(tc.tile_pool(name="wuvp", bufs=1))
        es_v = ExitStack()
        p_v = es_v.enter_context(tc.tile_pool(name="vp", bufs=1, side="right"))
        wuv_t = []
        for kt in range(KT3):
            t = p_wuv.tile([P, HDL], F32R, tag=f"wuv{kt}", name=f"wuv{kt}")
            nc.sync.dma_start(t[:], wuv[kt * P:(kt + 1) * P, :])
            wuv_t.append(t)
        vaug_t = [p_v.tile([P, HL * (DH + 1)], BF16, tag=f"v{i}", name=f"v{i}")
                  for i in range(KB)]
        for mt in range(KB):
            ps = p_ps_g.tile([P, N1], F32, tag="g", name=f"psv_{mt}")
            for kt in range(KT3):
                nc.tensor.matmul(ps[:], ckv_t[kt][:, mt * P:(mt + 1) * P],
                                 wuv_t[kt][:],
                                 start=(kt == 0), stop=(kt == KT3 - 1))
            va = vaug_t[mt].rearrange("p (h c) -> p h c", c=DH + 1)
            nc.vector.tensor_copy(va[:, :, 0:DH],
                                  ps.rearrange("p (h c) -> p h c", c=DH))
            nc.vector.memset(va[:, :, DH:DH + 1], 1.0)
        es_wuv.close()
        es_ckv.close()

        # ---------- phase 4: attention per head ----------
        p_ctx = es.enter_context(tc.tile_pool(name="ctxp", bufs=1))
        es_exp = ExitStack()
        p_e = es_exp.enter_context(tc.tile_pool(name="expp", bufs=34))
        es_sm = ExitStack()
        p_sm = es_sm.enter_context(tc.tile_pool(name="smallp", bufs=4))

        ctx_t = [p_ctx.tile([P, L], F32R, tag=f"ctxT{h}", name=f"ctxT{h}")
                 for h in range(HL)]
        for h in range(HL):
            for qch in range(NCH):
                exps = []
                for kb in range(KB):
                    ps = p_ps_sc.tile([P, N1], F32, tag="sc",
                                      name=f"sc_{h}_{qch}_{kb}")
                    nc.tensor.matmul(ps[:], kc_t[h][:, kb * P:(kb + 1) * P],
                                     qc_t[h][:, qch * N1:(qch + 1) * N1],
                                     start=True, stop=False)
                    nc.tensor.matmul(ps[:], kr_t[:, kb * P:(kb + 1) * P],
                                     qr_t[:, qch * N1:(qch + 1) * N1],
                                     start=False, stop=True)
                    et = p_e.tile([P, N1], BF16, tag="expT",
                                  name=f"et_{h}_{qch}_{kb}")
                    nc.scalar.activation(et[:], ps[:],
                                         mybir.ActivationFunctionType.Exp,
                                         bias=bias_t[:, kb:kb + 1], scale=SCALE)
                    exps.append(et)
                for qc in range(4):
                    q0 = qch * 4 + qc
                    pc = p_ps_av.tile([P, DH + 1], F32, tag="av",
                                      name=f"av_{h}_{q0}")
                    for kb in range(KB):
                        nc.tensor.matmul(
                            pc[:], exps[kb][:, qc * P:(qc + 1) * P],
                            vaug_t[kb][:, h * (DH + 1):(h + 1) * (DH + 1)],
                            start=(kb == 0), stop=(kb == KB - 1))
                    rc = p_sm.tile([P, 1], F32, tag="recip", name=f"rc_{h}_{q0}")
                    nc.vector.reciprocal(rc[:], pc[:, DH:DH + 1])
                    cn = p_sm.tile([P, DH], BF16, tag="cn", name=f"cn_{h}_{q0}")
                    nc.vector.tensor_scalar_mul(cn[:], pc[:, 0:DH], rc[:])
                    pt = p_ps_tp.tile([P, P], BF16, tag="tp", name=f"tp_{h}_{q0}")
                    nc.tensor.transpose(pt[:], cn[:], ident[:])
                    nc.vector.tensor_copy(ctx_t[h][:, q0 * P:(q0 + 1) * P], pt[:])
        es_sm.close()
        es_exp.close()
        es_v.close()
        es_kc.close()
        es_qc.close()
        es_krqr.close()

        # ---------- phase 5: partial out = ctx @ W_O[hg rows] ----------
        # W_O fully cached up front; one 1 MB store per q-row-block, issued
        # from the ACT HWDGE queue so loads (SP queue) don't contend.
        es_wo = ExitStack()
        p_wo = es_wo.enter_context(tc.tile_pool(name="wop", bufs=1))
        es_st = ExitStack()
        p_st = es_st.enter_context(tc.tile_pool(name="stagep", bufs=3))
        wo_t = {}
        for nci in range(NCH):
            for kt in range(HL):
                t = p_wo.tile([P, N1], F32R, tag=f"wo{nci}_{kt}",
                              name=f"wo_{nci}_{kt}")
                nc.sync.dma_start(t[:], wo[kt * P:(kt + 1) * P,
                                            nci * N1:(nci + 1) * N1])
                wo_t[(nci, kt)] = t
        for mt in range(KB):
            stg = p_st.tile([P, L], F32, tag="stage", name=f"st_{mt}")
            for nci in range(NCH):
                pool = p_ps_g if nci % 2 == 0 else p_ps_sc
                tag = "g" if nci % 2 == 0 else "sc"
                ps = pool.tile([P, N1], F32, tag=tag, name=f"ps5_{mt}_{nci}")
                for kt in range(HL):
                    nc.tensor.matmul(ps[:], ctx_t[kt][:, mt * P:(mt + 1) * P],
                                     wo_t[(nci, kt)][:],
                                     start=(kt == 0), stop=(kt == HL - 1))
                nc.vector.tensor_copy(stg[:, nci * N1:(nci + 1) * N1], ps[:])
            nc.scalar.dma_start(out_d[mt * P:(mt + 1) * P, :], stg[:])
        es_st.close()
        es_wo.close()

    nc.compile()
    return nc


_CACHE = {}


def _get_nc():
    if "nc" not in _CACHE:
        _CACHE["nc"] = build_nc()
    return _CACHE["nc"]


def _host_prep(x, attention_mask, W_DKV, W_DQ, W_UK, W_UV, W_UQ, W_KR, W_QR,
               W_O):
    f = np.float32
    x = np.asarray(x, f)
    attention_mask = np.asarray(attention_mask)
    W_DKV, W_DQ = np.asarray(W_DKV, f), np.asarray(W_DQ, f)
    W_UK, W_UV, W_UQ = np.asarray(W_UK, f), np.asarray(W_UV, f), np.asarray(W_UQ, f)
    W_KR, W_QR, W_O = np.asarray(W_KR, f), np.asarray(W_QR, f), np.asarray(W_O, f)

    perm = np.concatenate([np.arange(0, DH, 2), np.arange(1, DH, 2)])
    w1 = np.ascontiguousarray(
        np.concatenate([W_DKV, W_DQ, W_KR[:, perm]], axis=1))
    xTs = [np.ascontiguousarray(x[b].T) for b in range(B)]

    inv = 1.0 / (10000.0 ** (np.arange(0, DH, 2, dtype=f) / DH))
    freqs = np.arange(L, dtype=f)[:, None] * inv[None, :]
    rope = np.concatenate([np.sin(freqs), np.cos(freqs)], axis=-1).astype(f)
    s_tab, c_tab = rope[:, 0::2], rope[:, 1::2]
    sinT = np.ascontiguousarray(s_tab.T)
    cosT = np.ascontiguousarray(c_tab.T)

    maskbs = []
    for b in range(B):
        bias = np.where(attention_mask[b] == 0, f(NEG), f(0.0)).astype(f)
        maskbs.append(np.ascontiguousarray(bias.reshape(KB, P).T))

    in_maps = []
    for c in range(8):
        b, hg = c // HG, c % HG
        cols = slice(hg * HDL, (hg + 1) * HDL)
        in_maps.append({
            "xT": xTs[b],
            "w1": w1,
            "wuk": np.ascontiguousarray(W_UK[:, cols]),
            "w3q": np.ascontiguousarray(
                np.concatenate([W_UQ[:, cols], W_QR[:, perm]], axis=1)),
            "wuv": np.ascontiguousarray(W_UV[:, cols]),
            "wo": np.ascontiguousarray(W_O[hg * HDL:(hg + 1) * HDL, :]),
            "cosT": cosT,
            "sinT": sinT,
            "maskb": maskbs[b],
        })
    return in_maps


def kernel(x, attention_mask, W_DKV, W_DQ, W_UK, W_UV, W_UQ, W_KR, W_QR, W_O,
           **run_kwargs):
    in_maps = _host_prep(x, attention_mask, W_DKV, W_DQ, W_UK, W_UV, W_UQ,
                         W_KR, W_QR, W_O)
    nc = _get_nc()
    res = run_bass_kernel_spmd(nc, in_maps, core_ids=list(range(8)),
                               **run_kwargs)
    out = np.zeros((B, L, D), np.float32)
    for c in range(8):
        out[c // HG] += res.results[c]["out"]
    if run_kwargs:
        _CACHE["last_results"] = res
    return out


# revision 19
# speedup vs baseline: 1.5012x; 1.5012x over previous
"""MLA attention distributed over 8 TRN2 NeuronCores.

Sharding: tensor-parallel over heads (4 head-groups) x data-parallel over
batch (2). Each core computes, for its (batch, head-group):
  - the shared low-rank compressions c_kv/c_q and the rope key (replicated
    within a batch group),
  - K/V/Q up-projections for its 4 heads,
  - full attention for its 4 heads over all 2048 query positions,
  - a partial output projection (its heads' rows of W_O).
Host gather sums the 4 partial outputs per batch (row-parallel unshard).

All big GEMMs run on the PE in float32r (tf32-class precision, full speed at
N=512). Attention probabilities and V run in bf16; softmax is computed
without max-subtraction (scores are bounded ~|2| at this problem's scale) and
the denominator comes free from a ones-column appended to V.

Layout trick: everything downstream of x contracts over D, which must sit on
the partition dim, so the host feeds x[b].T. All projections are computed
directly in transposed layout [feature, seq]; rope is applied in transposed
layout using host-prepared sin/cos tables and an even/odd column permutation
baked into W_KR/W_QR.
"""

from contextlib import ExitStack

import numpy as np

import concourse.bacc as bacc
import concourse.mybir as mybir
import concourse.tile as tile
from concourse.bass_utils import run_bass_kernel_spmd
from concourse.masks import make_identity

B, L, D, H, DC, DH = 2, 2048, 2048, 16, 512, 128
HG = 4                 # head groups (tensor-parallel degree per batch)
HL = H // HG           # heads per core
HDL = HL * DH          # 512 head-dims per core
P = 128
N1 = 512               # matmul free-dim chunk
F32 = mybir.dt.float32
BF16 = mybir.dt.bfloat16
F32R = mybir.dt.float32r
SCALE = 1.0 / float(np.sqrt(2 * DH))
M1 = 2 * DC + DH       # 1152: [W_DKV | W_DQ | W_KR] fused output rows
MT1 = M1 // P          # 9
KT1 = D // P           # 16
NCH = L // N1          # 4
KT3 = DC // P          # 4
KB = L // P            # 16 key blocks
NEG = -30000.0         # additive mask bias for masked-out keys


def build_nc(debug=False):
    nc = bacc.Bacc(None, target_bir_lowering=False)

    xT = nc.dram_tensor("xT", [D, L], F32R, kind="ExternalInput")
    w1 = nc.dram_tensor("w1", [D, M1], F32R, kind="ExternalInput")
    wuk = nc.dram_tensor("wuk", [DC, HDL], F32R, kind="ExternalInput")
    w3q = nc.dram_tensor("w3q", [DC, HDL + DH], F32R, kind="ExternalInput")
    wuv = nc.dram_tensor("wuv", [DC, HDL], F32R, kind="ExternalInput")
    wo = nc.dram_tensor("wo", [HDL, D], F32R, kind="ExternalInput")
    cos_d = nc.dram_tensor("cosT", [DH // 2, L], F32, kind="ExternalInput")
    sin_d = nc.dram_tensor("sinT", [DH // 2, L], F32, kind="ExternalInput")
    mask_d = nc.dram_tensor("maskb", [P, KB], F32, kind="ExternalInput")
    out_d = nc.dram_tensor("out", [L, D], F32, kind="ExternalOutput")
    if debug:
        dbg = {n: nc.dram_tensor(f"dbg_{n}", [HL * P, L], F32,
                                 kind="ExternalOutput")
               for n in ("kc", "qc", "ctx")}
        dbg["kr"] = nc.dram_tensor("dbg_kr", [P, L], F32, kind="ExternalOutput")
        dbg["qr"] = nc.dram_tensor("dbg_qr", [P, L], F32, kind="ExternalOutput")
        dbg["v"] = nc.dram_tensor("dbg_v", [KB * P, HL * (DH + 1)], F32,
                                  kind="ExternalOutput")

    with tile.TileContext(nc) as tc, ExitStack() as es:
        # ---------- constant + psum pools (live whole kernel) ----------
        p_const = es.enter_context(tc.tile_pool(name="const", bufs=1))
        p_ps_g = es.enter_context(tc.tile_pool(name="psg", bufs=2, space="PSUM"))
        p_ps_sc = es.enter_context(tc.tile_pool(name="pssc", bufs=3, space="PSUM"))
        p_ps_av = es.enter_context(tc.tile_pool(name="psav", bufs=2, space="PSUM"))
        p_ps_tp = es.enter_context(tc.tile_pool(name="pstp", bufs=1, space="PSUM"))

        p_tab = es.enter_context(tc.tile_pool(name="tabp", bufs=1, side="right"))
        p_rope = es.enter_context(tc.tile_pool(name="ropep", bufs=1, side="right"))
        cos_t = p_tab.tile([DH // 2, L], F32, name="cos_t")
        sin_t = p_tab.tile([DH // 2, L], F32, name="sin_t")
        nc.sync.dma_start(cos_t[:], cos_d[:])
        nc.sync.dma_start(sin_t[:], sin_d[:])
        bias_t = p_const.tile([P, KB], F32, name="bias_t")
        nc.sync.dma_start(bias_t[:], mask_d[:])
        ident = p_const.tile([P, P], BF16, name="ident")
        make_identity(nc, ident[:])

        # ---------- phase-1 residents ----------
        es_ckv = ExitStack()
        p_ckv = es_ckv.enter_context(tc.tile_pool(name="ckvp", bufs=1))
        es_cq = ExitStack()
        p_cq = es_cq.enter_context(tc.tile_pool(name="cqp", bufs=1))
        es_xrk = ExitStack()
        p_xrk = es_xrk.enter_context(tc.tile_pool(name="xrkp", bufs=1))

        ckv_t = [p_ckv.tile([P, L], F32R, name=f"ckv{i}", tag=f"ckv{i}")
                 for i in range(KT3)]
        cq_t = [p_cq.tile([P, L], F32R, name=f"cq{i}", tag=f"cq{i}")
                for i in range(KT3)]
        xrk_t = p_xrk.tile([P, L], F32, name="xrkT")

        # ---------- phase 1: c_kvT | c_qT | xrkT = [Wdkv|Wdq|Wkr].T @ x.T ----
        es_w1 = ExitStack()
        p_w1 = es_w1.enter_context(tc.tile_pool(name="w1p", bufs=1))
        es_xn = ExitStack()
        p_xn = es_xn.enter_context(tc.tile_pool(name="xnp", bufs=16))

        # interleave the first x-chunk's DMAs with the weight-cache DMAs so
        # the PE k-loop can start as soon as (w1_0, xn_0_0) land
        w1_t = []
        xts0 = []
        for kt in range(KT1):
            t = p_xn.tile([P, N1], F32R, tag="xn", name=f"xn_0_{kt}")
            nc.sync.dma_start(t[:], xT[kt * P:(kt + 1) * P, 0:N1])
            xts0.append(t)
            t = p_w1.tile([P, M1], F32R, name=f"w1_{kt}", tag=f"w1_{kt}")
            nc.sync.dma_start(t[:], w1[kt * P:(kt + 1) * P, :])
            w1_t.append(t)

        dest1 = ckv_t + cq_t + [xrk_t]
        for nci in range(NCH):
            if nci == 0:
                xts = xts0
            else:
                xts = []
                for kt in range(KT1):
                    t = p_xn.tile([P, N1], F32R, tag="xn", name=f"xn_{nci}_{kt}")
                    nc.sync.dma_start(t[:], xT[kt * P:(kt + 1) * P,
                                                nci * N1:(nci + 1) * N1])
                    xts.append(t)
            for mt in range(MT1):
                ps = p_ps_g.tile([P, N1], F32, tag="g", name=f"ps1_{nci}_{mt}")
                for kt in range(KT1):
                    nc.tensor.matmul(ps[:], w1_t[kt][:, mt * P:(mt + 1) * P],
                                     xts[kt][:],
                                     start=(kt == 0), stop=(kt == KT1 - 1))
                nc.vector.tensor_copy(dest1[mt][:, nci * N1:(nci + 1) * N1], ps[:])
        es_xn.close()
        es_w1.close()

        # ---------- long-lived attention inputs (right-side stack) ----------
        es_krqr = ExitStack()
        p_krqr = es_krqr.enter_context(tc.tile_pool(name="krqrp", bufs=1, side="right"))
        kr_t = p_krqr.tile([P, L], F32R, name="krT", tag="krT")
        qr_t = p_krqr.tile([P, L], F32R, name="qrT", tag="qrT")
        es_qc = ExitStack()
        p_qc = es_qc.enter_context(tc.tile_pool(name="qcp", bufs=1, side="right"))
        qc_t = [p_qc.tile([P, L], F32R, tag=f"qc{i}", name=f"qc{i}")
                for i in range(HL)]

        # rope in transposed layout, emitted after the next GEMM phase so
        # its DVE work drains behind that phase's psum evictions.
        def rope_T(src_t, dst_t, pfx):
            # src rows 0:64 = even components, 64:128 = odd (host permuted W)
            for ch in range(NCH):
                cs = slice(ch * N1, (ch + 1) * N1)
                xo = p_rope.tile([64, N1], F32, tag="rxo", name=f"{pfx}xo{ch}")
                nc.sync.dma_start(xo[:], src_t[64:128, cs])
                t1 = p_rope.tile([64, N1], F32, tag="rt1", name=f"{pfx}t1{ch}")
                t2 = p_rope.tile([64, N1], F32, tag="rt2", name=f"{pfx}t2{ch}")
                h2 = p_rope.tile([64, N1], F32R, tag="rh2", name=f"{pfx}h2{ch}")
                xe = src_t[0:64, cs]
                cc, ss = cos_t[:, cs], sin_t[:, cs]
                nc.vector.tensor_tensor(t1[:], xe, cc, mybir.AluOpType.mult)
                nc.vector.tensor_tensor(t2[:], xo[:], ss, mybir.AluOpType.mult)
                nc.vector.tensor_tensor(dst_t[0:64, cs], t1[:], t2[:],
                                        mybir.AluOpType.subtract)
                t3 = p_rope.tile([64, N1], F32, tag="rt1", name=f"{pfx}t3{ch}")
                t4 = p_rope.tile([64, N1], F32, tag="rt2", name=f"{pfx}t4{ch}")
                nc.vector.tensor_tensor(t3[:], xe, ss, mybir.AluOpType.mult)
                nc.vector.tensor_tensor(t4[:], xo[:], cc, mybir.AluOpType.mult)
                nc.vector.tensor_tensor(h2[:], t3[:], t4[:], mybir.AluOpType.add)
                nc.sync.dma_start(dst_t[64:128, cs], h2[:])

        # ---------- phase 3q: q_cT | xrqT = [Wuq_hg|Wqr].T @ c_qT ----------
        es_w3q = ExitStack()
        p_w3q = es_w3q.enter_context(tc.tile_pool(name="w3qp", bufs=1))
        es_xrq = ExitStack()
        p_xrq = es_xrq.enter_context(tc.tile_pool(name="xrqp", bufs=1, side="right"))

        w3q_t = []
        for kt in range(KT3):
            t = p_w3q.tile([P, HDL + DH], F32R, tag=f"w3q{kt}", name=f"w3q{kt}")
            nc.sync.dma_start(t[:], w3q[kt * P:(kt + 1) * P, :])
            w3q_t.append(t)
        xrq_t = p_xrq.tile([P, L], F32, name="xrqT")
        dest3 = qc_t + [xrq_t]
        for nci in range(NCH):
            for mt in range(HL + 1):
                ps = p_ps_g.tile([P, N1], F32, tag="g", name=f"ps3_{nci}_{mt}")
                for kt in range(KT3):
                    nc.tensor.matmul(ps[:], w3q_t[kt][:, mt * P:(mt + 1) * P],
                                     cq_t[kt][:, nci * N1:(nci + 1) * N1],
                                     start=(kt == 0), stop=(kt == KT3 - 1))
                nc.vector.tensor_copy(dest3[mt][:, nci * N1:(nci + 1) * N1], ps[:])
        es_w3q.close()

        # rope-k, emitted here so its DVE work overlaps the 3k matmuls
        rope_T(xrk_t, kr_t, "k")
        es_xrk.close()
        es_cq.close()

        # rope-q, emitted here so its DVE work overlaps the 3k/3v matmuls
        rope_T(xrq_t, qr_t, "q")
        es_xrq.close()

        # ---------- phase 3k: k_cT = Wuk_hg.T @ c_kvT ----------
        es_kc = ExitStack()
        p_kc = es_kc.enter_context(tc.tile_pool(name="kcp", bufs=1, side="right"))
        kc_t = [p_kc.tile([P, L], F32R, tag=f"kc{i}", name=f"kc{i}")
                for i in range(HL)]
        es_wuk = ExitStack()
        p_wuk = es_wuk.enter_context(tc.tile_pool(name="wukp", bufs=1))
        wuk_t = []
        for kt in range(KT3):
            t = p_wuk.tile([P, HDL], F32R, tag=f"wuk{kt}", name=f"wuk{kt}")
            nc.sync.dma_start(t[:], wuk[kt * P:(kt + 1) * P, :])
            wuk_t.append(t)
        for nci in range(NCH):
            for mt in range(HL):
                ps = p_ps_g.tile([P, N1], F32, tag="g", name=f"ps3k_{nci}_{mt}")
                for kt in range(KT3):
                    nc.tensor.matmul(ps[:], wuk_t[kt][:, mt * P:(mt + 1) * P],
                                     ckv_t[kt][:, nci * N1:(nci + 1) * N1],
                                     start=(kt == 0), stop=(kt == KT3 - 1))
                nc.vector.tensor_copy(kc_t[mt][:, nci * N1:(nci + 1) * N1], ps[:])
        es_wuk.close()

        # ---------- phase 3v: v = c_kv @ Wuv_hg (natural), bf16 + ones col ---
        es_wuv = ExitStack()
        p_wuv = es_wuv.enter_context(tc.tile_pool(name="wuvp", bufs=1))
        es_v = ExitStack()
        p_v = es_v.enter_context(tc.tile_pool(name="vp", bufs=1, side="right"))
        wuv_t = []
        for kt in range(KT3):
            t = p_wuv.tile([P, HDL], F32R, tag=f"wuv{kt}", name=f"wuv{kt}")
            nc.sync.dma_start(t[:], wuv[kt * P:(kt + 1) * P, :])
            wuv_t.append(t)
        vaug_t = [p_v.tile([P, HL * (DH + 1)], BF16, tag=f"v{i}", name=f"v{i}")
                  for i in range(KB)]
        for mt in range(KB):
            ps = p_ps_g.tile([P, N1], F32, tag="g", name=f"psv_{mt}")
            for kt in range(KT3):
                nc.tensor.matmul(ps[:], ckv_t[kt][:, mt * P:(mt + 1) * P],
                                 wuv_t[kt][:],
                                 start=(kt == 0), stop=(kt == KT3 - 1))
            va = vaug_t[mt].rearrange("p (h c) -> p h c", c=DH + 1)
            nc.vector.tensor_copy(va[:, :, 0:DH],
                                  ps.rearrange("p (h c) -> p h c", c=DH))
            nc.vector.memset(va[:, :, DH:DH + 1], 1.0)
        es_wuv.close()
        es_ckv.close()

        # ---------- phase 4: attention per head ----------
        p_ctx = es.enter_context(tc.tile_pool(name="ctxp", bufs=1))
        es_exp = ExitStack()
        p_e = es_exp.enter_context(tc.tile_pool(name="expp", bufs=34))
        es_sm = ExitStack()
        p_sm = es_sm.enter_context(tc.tile_pool(name="smallp", bufs=6))

        ctx_t = [p_ctx.tile([P, L], F32R, tag=f"ctxT{h}", name=f"ctxT{h}")
                 for h in range(HL)]
        for h in range(HL):
            for qch in range(NCH):
                exps = []
                for kb in range(KB):
                    ps = p_ps_sc.tile([P, N1], F32, tag="sc",
                                      name=f"sc_{h}_{qch}_{kb}")
                    nc.tensor.matmul(ps[:], kc_t[h][:, kb * P:(kb + 1) * P],
                                     qc_t[h][:, qch * N1:(qch + 1) * N1],
                                     start=True, stop=False)
                    nc.tensor.matmul(ps[:], kr_t[:, kb * P:(kb + 1) * P],
                                     qr_t[:, qch * N1:(qch + 1) * N1],
                                     start=False, stop=True)
                    et = p_e.tile([P, N1], BF16, tag="expT",
                                  name=f"et_{h}_{qch}_{kb}")
                    nc.scalar.activation(et[:], ps[:],
                                         mybir.ActivationFunctionType.Exp,
                                         bias=bias_t[:, kb:kb + 1], scale=SCALE)
                    exps.append(et)
                for qc in range(4):
                    q0 = qch * 4 + qc
                    pc = p_ps_av.tile([P, DH + 1], F32, tag="av",
                                      name=f"av_{h}_{q0}")
                    for kb in range(KB):
                        nc.tensor.matmul(
                            pc[:], exps[kb][:, qc * P:(qc + 1) * P],
                            vaug_t[kb][:, h * (DH + 1):(h + 1) * (DH + 1)],
                            start=(kb == 0), stop=(kb == KB - 1))
                    rc = p_sm.tile([P, 1], F32, tag="recip", name=f"rc_{h}_{q0}")
                    nc.vector.reciprocal(rc[:], pc[:, DH:DH + 1])
                    cn = p_sm.tile([P, DH], BF16, tag="cn", name=f"cn_{h}_{q0}")
                    nc.vector.tensor_scalar_mul(cn[:], pc[:, 0:DH], rc[:])
                    pt = p_ps_tp.tile([P, P], BF16, tag="tp", name=f"tp_{h}_{q0}")
                    nc.tensor.transpose(pt[:], cn[:], ident[:])
                    nc.vector.tensor_copy(ctx_t[h][:, q0 * P:(q0 + 1) * P], pt[:])
        if debug:
            for i in range(HL):
                nc.gpsimd.dma_start(dbg["kc"][i * P:(i + 1) * P, :], kc_t[i][:])
                nc.gpsimd.dma_start(dbg["qc"][i * P:(i + 1) * P, :], qc_t[i][:])
                nc.gpsimd.dma_start(dbg["ctx"][i * P:(i + 1) * P, :], ctx_t[i][:])
            nc.gpsimd.dma_start(dbg["kr"][:], kr_t[:])
            nc.gpsimd.dma_start(dbg["qr"][:], qr_t[:])
            for i in range(KB):
                nc.gpsimd.dma_start(dbg["v"][i * P:(i + 1) * P, :], vaug_t[i][:])
        es_sm.close()
        es_exp.close()
        es_v.close()
        es_kc.close()
        es_qc.close()
        es_krqr.close()

        # ---------- phase 5: partial out = ctx @ W_O[hg rows] ----------
        # W_O fully cached up front; one 1 MB store per q-row-block, issued
        # from the ACT HWDGE queue so loads (SP queue) don't contend.
        es_wo = ExitStack()
        p_wo = es_wo.enter_context(tc.tile_pool(name="wop", bufs=1))
        es_st = ExitStack()
        p_st = es_st.enter_context(tc.tile_pool(name="stagep", bufs=3))
        wo_t = {}
        for nci in range(NCH):
            for kt in range(HL):
                t = p_wo.tile([P, N1], F32R, tag=f"wo{nci}_{kt}",
                              name=f"wo_{nci}_{kt}")
                nc.sync.dma_start(t[:], wo[kt * P:(kt + 1) * P,
                                            nci * N1:(nci + 1) * N1])
                wo_t[(nci, kt)] = t
        for mt in range(KB):
            stg = p_st.tile([P, L], F32, tag="stage", name=f"st_{mt}")
            for nci in range(NCH):
                pool = p_ps_g if nci % 2 == 0 else p_ps_sc
                tag = "g" if nci % 2 == 0 else "sc"
                ps = pool.tile([P, N1], F32, tag=tag, name=f"ps5_{mt}_{nci}")
                for kt in range(HL):
                    nc.tensor.matmul(ps[:], ctx_t[kt][:, mt * P:(mt + 1) * P],
                                     wo_t[(nci, kt)][:],
                                     start=(kt == 0), stop=(kt == HL - 1))
                nc.vector.tensor_copy(stg[:, nci * N1:(nci + 1) * N1], ps[:])
            nc.scalar.dma_start(out_d[mt * P:(mt + 1) * P, :], stg[:])
        es_st.close()
        es_wo.close()

    nc.compile()
    return nc


_CACHE = {}


def _get_nc():
    if "nc" not in _CACHE:
        _CACHE["nc"] = build_nc()
    return _CACHE["nc"]


def _host_prep(x, attention_mask, W_DKV, W_DQ, W_UK, W_UV, W_UQ, W_KR, W_QR,
               W_O):
    f = np.float32
    x = np.asarray(x, f)
    attention_mask = np.asarray(attention_mask)
    W_DKV, W_DQ = np.asarray(W_DKV, f), np.asarray(W_DQ, f)
    W_UK, W_UV, W_UQ = np.asarray(W_UK, f), np.asarray(W_UV, f), np.asarray(W_UQ, f)
    W_KR, W_QR, W_O = np.asarray(W_KR, f), np.asarray(W_QR, f), np.asarray(W_O, f)

    perm = np.concatenate([np.arange(0, DH, 2), np.arange(1, DH, 2)])
    w1 = np.ascontiguousarray(
        np.concatenate([W_DKV, W_DQ, W_KR[:, perm]], axis=1))
    xTs = [np.ascontiguousarray(x[b].T) for b in range(B)]

    inv = 1.0 / (10000.0 ** (np.arange(0, DH, 2, dtype=f) / DH))
    freqs = np.arange(L, dtype=f)[:, None] * inv[None, :]
    rope = np.concatenate([np.sin(freqs), np.cos(freqs)], axis=-1).astype(f)
    s_tab, c_tab = rope[:, 0::2], rope[:, 1::2]
    sinT = np.ascontiguousarray(s_tab.T)
    cosT = np.ascontiguousarray(c_tab.T)

    maskbs = []
    for b in range(B):
        bias = np.where(attention_mask[b] == 0, f(NEG), f(0.0)).astype(f)
        maskbs.append(np.ascontiguousarray(bias.reshape(KB, P).T))

    in_maps = []
    for c in range(8):
        b, hg = c // HG, c % HG
        cols = slice(hg * HDL, (hg + 1) * HDL)
        in_maps.append({
            "xT": xTs[b],
            "w1": w1,
            "wuk": np.ascontiguousarray(W_UK[:, cols]),
            "w3q": np.ascontiguousarray(
                np.concatenate([W_UQ[:, cols], W_QR[:, perm]], axis=1)),
            "wuv": np.ascontiguousarray(W_UV[:, cols]),
            "wo": np.ascontiguousarray(W_O[hg * HDL:(hg + 1) * HDL, :]),
            "cosT": cosT,
            "sinT": sinT,
            "maskb": maskbs[b],
        })
    return in_maps


def kernel(x, attention_mask, W_DKV, W_DQ, W_UK, W_UV, W_UQ, W_KR, W_QR, W_O,
           **run_kwargs):
    in_maps = _host_prep(x, attention_mask, W_DKV, W_DQ, W_UK, W_UV, W_UQ,
                         W_KR, W_QR, W_O)
    nc = _get_nc()
    res = run_bass_kernel_spmd(nc, in_maps, core_ids=list(range(8)),
                               **run_kwargs)
    out = np.zeros((B, L, D), np.float32)
    for c in range(8):
        out[c // HG] += res.results[c]["out"]
    if run_kwargs:
        _CACHE["last_results"] = res
    return out
